# revision 2
# baseline (speedup 1.0000x reference)
"""Trainium2 Bass kernel for BaseSpectrogram1D.

x[128, 131072] -> |DFT(window * overlapping_frames(x - mean))| [128, 511, 257]

Sharding: pure data parallel, batch dim split across 8 NeuronCores
(16 rows each). window/fourier_matrix are combined host-side into one
real [512, 512] matrix (Re bins 0..256 | Im bins 1..255; Im[0] and
Im[256] are exactly zero and omitted so a frame-tile's whole DFT fits a
single 512-wide PSUM bank).

Per core:
  - DMA x chunk-major (128-sample chunks on partitions)
  - row-sums (DVE) + ones-matmul -> per-batch mean, broadcast to all
    partitions via a [128,128] ones matmul
  - PE transpose 128x128 blocks (exact), ACT copy-back with fused
    mean subtract, producing the fp32r frames^T layout
  - per 128-frame tile: 4 accumulated fp32r matmuls (K=4x128) against
    the combined DFT matrix -> PSUM [128, 512]
  - Square (DVE) -> paired add re^2+im^2 -> Sqrt (ACT) -> DMA out
"""

import sys

if "/opt/trn_rl_repo" not in sys.path:
    sys.path.insert(0, "/opt/trn_rl_repo")

import numpy as np

L = 131072
B = 128
N = 512  # frame length
M = 511  # frames
STRIDE = 256
KH = 257  # one-sided bins
NCORES = 8
BPC = B // NCORES  # batches per core = 16
NBLK = L // (128 * 128)  # 128x128 transpose blocks per batch = 8

_CACHE = {}


def _tukey(n_pts, alpha=0.25):
    n = np.arange(n_pts, dtype=np.float64)
    edge = alpha * (n_pts - 1) / 2.0
    w = np.ones(n_pts)
    left = n < edge
    w[left] = 0.5 * (1.0 + np.cos(np.pi * (2.0 * n[left] / (alpha * (n_pts - 1)) - 1.0)))
    right = n > (n_pts - 1) - edge
    w[right] = 0.5 * (
        1.0 + np.cos(np.pi * (2.0 * n[right] / (alpha * (n_pts - 1)) - 2.0 / alpha + 1.0))
    )
    return w


def _default_consts():
    w = _tukey(N, 0.25)
    w = (w / w.sum()).astype(np.float32)
    nk = np.outer(np.arange(N, dtype=np.float64), np.arange(N, dtype=np.float64))
    sigma = np.exp(-2j * np.pi / N)
    fm = (sigma**nk)[:, :KH] * np.sqrt(N)
    return w, fm.astype(np.complex64)


def _build():
    """Build + schedule the Bass module once per process."""
    if "nc" in _CACHE:
        return _CACHE["nc"]

    import concourse.bass as bass
    import concourse.mybir as mybir
    import concourse.tile as tile
    from concourse import bacc

    F32 = mybir.dt.float32
    F32R = mybir.dt.float32r
    AF = mybir.ActivationFunctionType

    nc = bacc.Bacc(trn_type="TRN2", target_bir_lowering=False, debug=False)

    x_d = nc.dram_tensor("x", [BPC, L], F32, kind="ExternalInput").ap()
    wfm_d = nc.dram_tensor("wfm", [4, 128, N], F32R, kind="ExternalInput").ap()
    id_d = nc.dram_tensor("ident", [128, 128], F32, kind="ExternalInput").ap()
    out_d = nc.dram_tensor("out", [BPC, M, KH], F32, kind="ExternalOutput").ap()

    with tile.TileContext(nc) as tc:
        with (
            tc.tile_pool(name="consts", bufs=1) as consts,
            tc.tile_pool(name="nat", bufs=3) as natp,
            tc.tile_pool(name="xt", bufs=2) as xtp,
            tc.tile_pool(name="small", bufs=4) as smallp,
            tc.tile_pool(name="sq", bufs=3) as sqp,
            tc.tile_pool(name="mag", bufs=4) as magp,
            tc.tile_pool(name="pmu", bufs=2, space="PSUM") as pmup,
            tc.tile_pool(name="ptp", bufs=4, space="PSUM") as ptpp,
            tc.tile_pool(name="pspec", bufs=2, space="PSUM") as pspecp,
        ):
            wfm_s = consts.tile([128, 4, N], F32R)
            ident = consts.tile([128, 128], F32)
            ones = consts.tile([128, 128], F32)
            nc.sync.dma_start(out=wfm_s, in_=wfm_d.rearrange("j p n -> p j n"))
            nc.sync.dma_start(out=ident, in_=id_d)
            nc.vector.memset(ones, 1.0)

            for b in range(BPC):
                # chunk-major load: nat[q, t, e] = x[b, 16384*t + 128*q + e]
                nat = natp.tile([128, NBLK, 128], F32)
                nc.sync.dma_start(
                    out=nat,
                    in_=x_d[b].rearrange("(t q e) -> q t e", t=NBLK, q=128),
                )

                # per-partition row sums, then all-partition total via ones
                # matmul (every output partition gets the full sum)
                part = smallp.tile([128, 1], F32)
                nc.vector.reduce_sum(
                    part, nat.rearrange("q t e -> q (t e)"), axis=mybir.AxisListType.X
                )
                mu_ps = pmup.tile([128, 1], F32)
                nc.tensor.matmul(mu_ps, ones, part, start=True, stop=True)
                negmu = smallp.tile([128, 1], F32)
                nc.scalar.activation(negmu, mu_ps, AF.Copy, scale=-1.0 / L)

                # transpose blocks + mean-subtracting fp32r copy-back
                xt = xtp.tile([128, 1024], F32R)
                xt3 = xt.rearrange("p (c two) -> p c two", two=2)
                for t in range(NBLK):
                    tp_ps = ptpp.tile([128, 128], F32)
                    nc.tensor.transpose(tp_ps, nat[:, t, :], ident)
                    nc.scalar.activation(
                        xt[:, t * 128 : (t + 1) * 128],
                        tp_ps,
                        AF.Identity,
                        bias=negmu,
                    )

                # frame-tile matmuls + magnitude epilogue
                for mt in range(4):
                    m0 = mt * 128
                    mm = min(128, M - m0)
                    spec = pspecp.tile([128, N], F32)
                    for j in range(4):
                        lhsT = xt3[:, m0 + j // 2 : m0 + j // 2 + mm, j % 2]
                        nc.tensor.matmul(
                            spec[:mm],
                            lhsT,
                            wfm_s[:, j],
                            start=(j == 0),
                            stop=(j == 3),
                        )
                    sq = sqp.tile([128, N], F32)
                    nc.scalar.activation(sq[:mm], spec[:mm], AF.Square)
                    magsq = magp.tile([128, KH], F32)
                    nc.vector.tensor_add(
                        magsq[:mm, 1:256], sq[:mm, 1:256], sq[:mm, 257:512]
                    )
                    nc.vector.tensor_copy(
                        magsq[:mm, 0:257:256], sq[:mm, 0:257:256]
                    )
                    mag = magp.tile([128, KH], F32)
                    nc.scalar.activation(mag[:mm], magsq[:mm], AF.Sqrt)
                    nc.sync.dma_start(out=out_d[b, m0 : m0 + mm, :], in_=mag[:mm])

    nc.compile()
    _CACHE["nc"] = nc
    return nc


def kernel(x, window=None, fourier_matrix=None, **_unused):
    from concourse.bass_utils import run_bass_kernel_spmd

    x = np.ascontiguousarray(np.asarray(x, dtype=np.float32))
    assert x.shape == (B, L)

    if window is None or fourier_matrix is None:
        window, fourier_matrix = _default_consts()
    window = np.asarray(window)
    fourier_matrix = np.asarray(fourier_matrix)

    wfm = fourier_matrix.astype(np.complex64) * window.astype(np.float32)[:, None]
    wfm_cat = np.concatenate(
        [wfm.real[:, 0:257], wfm.imag[:, 1:256]], axis=1
    ).astype(np.float32)  # [512, 512]
    wfm_in = np.ascontiguousarray(wfm_cat.reshape(4, 128, N))
    ident = np.eye(128, dtype=np.float32)

    nc = _build()
    in_maps = [
        {"x": x[i * BPC : (i + 1) * BPC], "wfm": wfm_in, "ident": ident}
        for i in range(NCORES)
    ]
    res = run_bass_kernel_spmd(nc, in_maps, core_ids=list(range(NCORES)))
    return np.concatenate([r["out"] for r in res.results], axis=0)


if __name__ == "__main__":
    rng = np.random.default_rng(0)
    x = rng.standard_normal((B, L)).astype(np.float32)
    out = kernel(x)
    print("out", out.shape, out.dtype, float(out.max()))
